# revision 9
# baseline (speedup 1.0000x reference)
"""Ewald realspace potential on 8 Trainium2 NeuronCores — symmetric version.

pot = sum_ij erf(|r_ij|/sqrt(2))/(|r_ij|+1e-6) * (q_i . q_j) / (4*pi)
      + sum(q^2) / (2*pi)^1.5

The pairwise kernel is symmetric, so each unordered 128x128 block pair is
computed exactly once — half the engine work of the row-tiled baseline.

Partitioning (SPMD-uniform, balanced):
  - Atoms are ordered by reverse Cuthill-McKee on the near-pair graph
    (r < CUT), so near pairs live in a narrow diagonal band of the block
    matrix (bandwidth ~4 blocks of 128).
  - 64 row blocks of 128; core c owns the 8 blocks g with g % 8 == c
    (interleaved). Block pair (a, b) with d = (b - a) mod 64 is computed
    by the core owning a iff d in {1..31}, d == 0 (diag), or d == 32 and
    (a div 8) < 4.  Each core gets exactly 260 block pairs.
  - Per core: 64 column positions p (lhs = all 8192 atoms rolled by
    c*128; position p holds global block (c+p) % 64).  Position p needs a
    CONTIGUOUS window of 4 local row blocks (5 for p in {32,40,48,56}),
    identical across cores.  The kernel computes d2[j=128, i=window] via
    an augmented f32r matmul (Dekker hi/lo split, 13 K-rows — K is free),
    then u = rsqrt(d2+1e-6) on ACT (bf16 out), and for the few near
    sub-blocks (window tails, thanks to RCM banding) w = d2*u, e =
    erf(w/sqrt2), kern = e*u.  Far sub-blocks use kern = u exactly
    (erf saturates to 1.0f beyond r~4.3; classification cut 3.0 is safe
    at rel-err ~4e-5).  All staged values bf16 (~4e-4 pot error, budget
    is 2e-2).
  - Reduce: F[32, 512] PSUM accumulates ALL positions via q stationaries
    [128, 32] zero-padded per window group s (8 groups x 4 channels), so
    a single accumulation region at partition 0 suffices.  Window-ext
    columns (5-block positions) go to F_ext[32, 128].  Diagonal blocks
    are kern-scaled by 0.5 (and diag elements masked to 0) so the host
    can uniformly double: pot = sum q_i.F_i / (2*pi) + self.
  - ACT table discipline: one rsqrt phase then one erf phase (2 table
    loads).  Near-tail w values are staged into contiguous bf16 arenas so
    the erf phase is a handful of wide instructions.
"""

import numpy as np

N = 8192
C = 4
NCORES = 8
JCH = 128                 # atoms per block (partition dim)
NB = 64                   # global 128-blocks
NLB = 8                   # local row blocks per core
NI = 1024                 # rows per core
CUT = 3.0                 # near-pair cut for sort + erf classification
RSQRT_BIAS = 1e-6
SQRT1_2 = float(1.0 / np.sqrt(2.0))
TWOPI = 2.0 * np.pi
ARENA_MAX = 8192          # max erf-arena columns per table-phase batch
NECHUNK = 4               # erf instructions per batch (pipelining)

_cache = {}


def _window_table():
    """Static per-position window: (s, w). Window rows are local blocks
    (s+k) % 8 for k in 0..w-1; the diagonal block, when present
    (p % 8 == 0), is always the LAST window block."""
    wins = []
    for p in range(64):
        rows = [
            r for r in range(8)
            if ((p - 8 * r) % 64) <= 31 or (((p - 8 * r) % 64) == 32 and r < 4)
        ]
        w = len(rows)
        rset = set(rows)
        s = next(
            cand for cand in range(8)
            if all(((cand + k) % 8) in rset for k in range(w))
        )
        if p % 8 == 0:
            assert (s + w - 1) % 8 == (p // 8) % 8
        wins.append((s, w))
    return wins


WINDOWS = _window_table()


def _split10(x):
    """Split f32 array into hi (10-bit mantissa, exact under f32r) + lo."""
    x = np.ascontiguousarray(x, dtype=np.float32)
    b = x.view(np.int32) & np.int32(~0x3FFF)
    hi = b.view(np.float32)
    return hi, (x - hi).astype(np.float32)


def _near_pairs(p64):
    """All index pairs (ii, jj), ii<jj, with |p_i - p_j| < CUT."""
    pn = (p64 ** 2).sum(1)
    out_i, out_j = [], []
    for a0 in range(0, N, 1024):
        d2 = pn[a0:a0 + 1024, None] + pn[None, :] - 2.0 * (p64[a0:a0 + 1024] @ p64.T)
        ii, jj = np.nonzero(d2 < CUT * CUT)
        ii = ii + a0
        keep = ii < jj
        out_i.append(ii[keep])
        out_j.append(jj[keep])
    return np.concatenate(out_i), np.concatenate(out_j)


def _rcm_order(p64):
    """Reverse Cuthill-McKee ordering of the near-pair graph (bandwidth
    minimization -> near block pairs concentrate at small block-index
    distance).  scipy if available, else a deterministic numpy BFS RCM."""
    ii, jj = _near_pairs(p64)
    try:
        import scipy.sparse as sp
        from scipy.sparse.csgraph import reverse_cuthill_mckee

        g = sp.csr_matrix(
            (np.ones(len(ii), np.int8), (ii, jj)), shape=(N, N)
        )
        g = g + g.T
        return np.asarray(reverse_cuthill_mckee(g, symmetric_mode=True), np.int64)
    except Exception:
        pass
    # numpy RCM fallback
    order = np.argsort(np.concatenate([ii, jj]), kind="stable")
    src = np.concatenate([ii, jj])[order]
    dst = np.concatenate([jj, ii])[order]
    deg = np.bincount(src, minlength=N)
    starts = np.zeros(N + 1, np.int64)
    np.cumsum(deg, out=starts[1:])
    visited = np.zeros(N, bool)
    out = []
    remaining = set(range(N))
    while remaining:
        root = min(remaining, key=lambda v: (deg[v], v))
        visited[root] = True
        remaining.discard(root)
        queue = [root]
        out.append(root)
        qi = 0
        while qi < len(queue):
            v = queue[qi]
            qi += 1
            nbrs = dst[starts[v]:starts[v + 1]]
            nbrs = [u for u in nbrs.tolist() if not visited[u]]
            nbrs.sort(key=lambda u: (deg[u], u))
            for u in nbrs:
                if not visited[u]:
                    visited[u] = True
                    remaining.discard(u)
                    queue.append(u)
                    out.append(u)
    return np.asarray(out[::-1], np.int64)


def _sort_and_flags(positions):
    """RCM atom order + per-position erf tail start (block index, -1 if the
    position needs no erf at all).  A window sub-block (p, k) is near iff
    ANY core's corresponding global block pair has a pair under CUT (the
    SPMD program is shared, so flags are the union over cores)."""
    p64 = np.asarray(positions, np.float64)
    perm = _rcm_order(p64)
    ps = p64[perm]
    pn = (ps ** 2).sum(1)
    B = np.zeros((NB, NB), dtype=bool)
    for a0 in range(0, N, 1024):
        d2 = pn[a0:a0 + 1024, None] + pn[None, :] - 2.0 * (ps[a0:a0 + 1024] @ ps.T)
        nb = (d2 < CUT * CUT).reshape(8, JCH, NB, JCH).any(axis=(1, 3))
        B[a0 // JCH: a0 // JCH + 8] |= nb
    B |= B.T
    fl_k0 = []
    for p in range(64):
        s, w = WINDOWS[p]
        ks = [
            k for k in range(w)
            if any(B[8 * ((s + k) % 8) + c, (c + p) % 64] for c in range(NCORES))
        ]
        fl_k0.append(min(ks) if ks else -1)
    # diagonal positions must always take the erf path (self-block pairs
    # are near by construction; guard against numeric edge cases)
    for p in range(0, 64, 8):
        s, w = WINDOWS[p]
        if fl_k0[p] < 0:
            fl_k0[p] = w - 1
    return perm, tuple(fl_k0)


def _schedule(meta):
    """Emission schedule: rsqrt pair items + erf chunk assignment.

    Returns (batches, n_ext_total). Each batch:
      items: list of position tuples (1 or 2 positions, same d2 tile)
      echunks: list of lists of flagged positions (one erf inst each)
    """
    ext = [p for p in range(64) if WINDOWS[p][1] == 5]
    reg = [p for p in range(64) if WINDOWS[p][1] == 4]
    regf = [p for p in reg if meta[p] >= 0]
    regu = [p for p in reg if meta[p] < 0]

    def fl_len(p):
        s, w = WINDOWS[p]
        return (w - meta[p]) * JCH if meta[p] >= 0 else 0

    # alternate flagged/unflagged positions, then group into triples so one
    # rsqrt instruction covers 3 positions (d2 tile [128, 1536] = 3 PSUM
    # banks); the 4 extended (640-wide) positions ride as solo items
    inter = []
    fi, ui = 0, 0
    for k in range(len(reg)):
        if (k % 2 == 0 and ui < len(regu)) or fi >= len(regf):
            inter.append(regu[ui]); ui += 1
        else:
            inter.append(regf[fi]); fi += 1
    items = [tuple(inter[i:i + 3]) for i in range(0, len(inter), 3)]
    sx = [(p,) for p in ext]
    step = max(1, len(items) // (len(sx) + 1))
    for i, it in enumerate(sx):
        items.insert(min(len(items), (i + 1) * step + i), it)

    # batches bounded by arena size
    batches = []
    cur, cur_arena = [], 0
    for it in items:
        alen = sum(fl_len(p) for p in it)
        if cur and cur_arena + alen > ARENA_MAX:
            batches.append(cur)
            cur, cur_arena = [], 0
        cur.append(it)
        cur_arena += alen
    if cur:
        batches.append(cur)

    out = []
    for bitems in batches:
        flagged = [p for it in bitems for p in it if meta[p] >= 0]
        total = sum(fl_len(p) for p in flagged)
        nch = min(NECHUNK, max(1, len(flagged)))
        target = max(1, (total + nch - 1) // nch)
        echunks, cur, acc = [], [], 0
        for p in flagged:
            cur.append(p)
            acc += fl_len(p)
            if acc >= target and len(echunks) < nch - 1:
                echunks.append(cur)
                cur, acc = [], 0
        if cur:
            echunks.append(cur)
        out.append((bitems, echunks))
    return out, len(ext)


def _build(meta):
    """meta: tuple of 64 ints — per-position erf tail start block (-1 = no
    erf; kern = rsqrt everywhere in that window)."""
    import concourse.bass as bass
    import concourse.mybir as mybir
    import concourse.tile as tile

    AF = mybir.ActivationFunctionType
    dt = mybir.dt
    ALU = mybir.AluOpType
    nc = bass.Bass(trn_type="TRN2")

    lhs = nc.dram_tensor("lhs", [13, N], dt.float32r, kind="ExternalInput")
    rhs = nc.dram_tensor("rhs", [13, 1536], dt.float32r, kind="ExternalInput")
    qT = nc.dram_tensor("qT", [JCH, NB * 32], dt.float32, kind="ExternalInput")
    dmask = nc.dram_tensor("dmask", [JCH, JCH], dt.float32, kind="ExternalInput")
    f_out = nc.dram_tensor("f_out", [32, 640], dt.float32, kind="ExternalOutput")

    def raw_act(out, in_, func, bias=0.0, scale=1.0):
        return nc.scalar.add_instruction(
            mybir.InstActivation(
                name=nc.get_next_instruction_name(),
                ins=[
                    nc.scalar.lower_ap(in_),
                    mybir.ImmediateValue(dtype=dt.float32, value=bias),
                    mybir.ImmediateValue(dtype=dt.float32, value=scale),
                    mybir.ImmediateValue(dtype=dt.float32, value=0.0),
                ],
                outs=[nc.scalar.lower_ap(out)],
                func=func,
            )
        )

    batches, n_ext_total = _schedule(meta)

    with tile.TileContext(nc) as tc:
        with (
            tc.tile_pool(name="const", bufs=1) as cpool,
            tc.tile_pool(name="u", bufs=1) as upool,
            tc.tile_pool(name="wk", bufs=1) as wpool,
            tc.tile_pool(name="d2", bufs=2, space="PSUM") as d2pool,
            tc.tile_pool(name="facc", bufs=1, space="PSUM") as fpool,
        ):
            lhs_t = cpool.tile([13, N], dt.float32r, tag="lhs")
            rhs_t = cpool.tile([13, 1536], dt.float32r, tag="rhs")
            qf_t = cpool.tile([JCH, NB * 32], dt.float32, tag="qT")
            qb_t = cpool.tile([JCH, NB * 32], dt.bfloat16, tag="qTb")
            m_t = cpool.tile([JCH, JCH], dt.float32, tag="dmask")
            # spread the big lhs load over the 3 DMA-capable engine queues
            # (SP, ACT, Pool), ordered so early positions' data lands first
            nc.scalar.dma_start(lhs_t[:, 0:1024], lhs[:, 0:1024])
            nc.sync.dma_start(rhs_t[:], rhs[:])
            nc.sync.dma_start(lhs_t[:, 1024:3072], lhs[:, 1024:3072])
            nc.gpsimd.dma_start(m_t[:], dmask[:])
            nc.gpsimd.dma_start(qf_t[:], qT[:])
            nc.gpsimd.dma_start(lhs_t[:, 3072:5120], lhs[:, 3072:5120])
            nc.gpsimd.dma_start(lhs_t[:, 5120:8192], lhs[:, 5120:8192])
            nc.vector.tensor_copy(qb_t[:], qf_t[:])

            f_all = fpool.tile([32, 640], dt.float32, tag="fa")

            n_main = [0]
            n_ext = [0]

            def reduce_pos(p, u_ap, W):
                nc.tensor.matmul(
                    f_all[:, 0:512],
                    qb_t[:, p * 32:(p + 1) * 32],
                    u_ap[:, 0:512],
                    start=(n_main[0] == 0),
                    stop=(n_main[0] == 63),
                )
                n_main[0] += 1
                if W > 512:
                    nc.tensor.matmul(
                        f_all[:, 512:512 + (W - 512)],
                        qb_t[:, p * 32:(p + 1) * 32],
                        u_ap[:, 512:W],
                        start=(n_ext[0] == 0),
                        stop=(n_ext[0] == n_ext_total - 1),
                    )
                    n_ext[0] += 1

            prev_last_erf = None
            uidx = [0]
            for bitems, echunks in batches:
                # ---- phase A: d2 matmuls + rsqrt (+ stage w for erf tails,
                # reduce erf-free positions) ----
                pos_u = {}       # p -> (u_ap slice, W)
                warena = {}      # p -> (w_tile, e_tile, offset)
                last_rsqrt = None
                # pre-alloc per-chunk w/e arenas
                chunk_tiles = []
                for ci, ch in enumerate(echunks):
                    clen = sum(
                        (WINDOWS[p][1] - meta[p]) * JCH for p in ch
                    )
                    w_t = wpool.tile([JCH, clen], dt.bfloat16,
                                     tag=f"w{uidx[0]}_{ci}")
                    e_t = wpool.tile([JCH, clen], dt.bfloat16,
                                     tag=f"e{uidx[0]}_{ci}")
                    off = 0
                    for p in ch:
                        warena[p] = (w_t, e_t, off)
                        off += (WINDOWS[p][1] - meta[p]) * JCH
                    chunk_tiles.append((w_t, e_t))

                for it in bitems:
                    Ws = [WINDOWS[p][1] * JCH for p in it]
                    tot = sum(Ws)
                    d2 = d2pool.tile([JCH, 1536], dt.float32, tag="d2")
                    off = 0
                    for p, W in zip(it, Ws):
                        s = WINDOWS[p][0]
                        for h0 in range(0, W, 512):
                            h1 = min(h0 + 512, W)
                            nc.tensor.matmul(
                                d2[:, off + h0:off + h1],
                                lhs_t[:, p * JCH:(p + 1) * JCH],
                                rhs_t[:, s * JCH + h0:s * JCH + h1],
                                start=True,
                                stop=True,
                            )
                        if p % 8 == 0:
                            sl = slice(off + W - JCH, off + W)
                            nc.vector.tensor_mul(d2[:, sl], d2[:, sl], m_t[:])
                        off += W
                    u_t = upool.tile([JCH, tot], dt.bfloat16,
                                     tag=f"u{uidx[0]}")
                    uidx[0] += 1
                    ri = raw_act(u_t[:], d2[:, 0:tot], AF.Rsqrt,
                                 bias=RSQRT_BIAS)
                    if prev_last_erf is not None:
                        tile.add_dep_helper(
                            ri.ins, prev_last_erf.ins, sync=False,
                            reason="ACT table phase ordering",
                        )
                        prev_last_erf = None
                    last_rsqrt = ri
                    off = 0
                    for p, W in zip(it, Ws):
                        u_ap = u_t[:, off:off + W]
                        pos_u[p] = (u_ap, W)
                        if meta[p] >= 0:
                            w_t, e_t, aoff = warena[p]
                            f0 = meta[p] * JCH
                            nc.vector.tensor_mul(
                                w_t[:, aoff:aoff + W - f0],
                                d2[:, off + f0:off + W],
                                u_ap[:, f0:W],
                            )
                        else:
                            reduce_pos(p, u_ap, W)
                        off += W

                # ---- phase B: erf + kern muls + remaining reduces ----
                nmul = [0]
                for ci, ch in enumerate(echunks):
                    w_t, e_t = chunk_tiles[ci]
                    ei = raw_act(e_t[:], w_t[:], AF.Erf, scale=SQRT1_2)
                    tile.add_dep_helper(
                        ei.ins, last_rsqrt.ins, sync=False,
                        reason="ACT table phase ordering (erf after rsqrt)",
                    )
                    prev_last_erf = ei
                    for p in ch:
                        u_ap, W = pos_u[p]
                        _, _, aoff = warena[p]
                        f0 = meta[p] * JCH
                        fl = W - f0
                        if p % 8 == 0:
                            if fl > JCH:
                                eng = nc.gpsimd if nmul[0] % 3 == 2 else nc.vector
                                nmul[0] += 1
                                eng.tensor_mul(
                                    u_ap[:, f0:W - JCH],
                                    e_t[:, aoff:aoff + fl - JCH],
                                    u_ap[:, f0:W - JCH],
                                )
                            # diagonal block: kern *= 0.5 so the host can
                            # uniformly double off-diagonal coverage
                            nc.vector.scalar_tensor_tensor(
                                u_ap[:, W - JCH:W],
                                e_t[:, aoff + fl - JCH:aoff + fl],
                                0.5,
                                u_ap[:, W - JCH:W],
                                ALU.mult,
                                ALU.mult,
                            )
                        else:
                            eng = nc.gpsimd if nmul[0] % 3 == 2 else nc.vector
                            nmul[0] += 1
                            eng.tensor_mul(
                                u_ap[:, f0:W],
                                e_t[:, aoff:aoff + fl],
                                u_ap[:, f0:W],
                            )
                        reduce_pos(p, u_ap, W)

            f_sb = cpool.tile([32, 640], dt.float32, tag="fsb")
            nc.vector.tensor_copy(f_sb[:], f_all[:])
            nc.sync.dma_start(f_out[:], f_sb[:])

    _split_excess_waits(nc)
    return nc


def _split_excess_waits(nc, limit=1):
    """This walrus build accepts at most one sync wait per instruction;
    split extras onto preceding single-wait NOPs on the same engine."""
    import concourse.mybir as mybir

    for f in nc.m.functions:
        for bb in f.blocks:
            new_insts = []
            for inst in bb.instructions:
                si = getattr(inst, "sync_info", None)
                if si is not None and si.on_wait and len(si.on_wait) > limit:
                    waits = list(si.on_wait)
                    extra, keep = waits[:-limit], waits[-limit:]
                    for k, w in enumerate(extra):
                        nop = mybir.InstNoOp(
                            name=f"{inst.name}-ws{k}",
                            ins=[],
                            outs=[],
                            engine=inst.engine,
                            sync_info=mybir.SyncInfo(on_wait=[w], on_update=[]),
                        )
                        nc.register_instruction(nop, overwrite=True)
                        new_insts.append(nop)
                    inst.sync_info = mybir.SyncInfo(
                        on_wait=keep, on_update=list(si.on_update)
                    )
                new_insts.append(inst)
            bb.instructions[:] = new_insts


def _host_inputs(positions, q, perm):
    """Per-core input dicts for the symmetric layout."""
    positions = np.asarray(positions, np.float32)[perm]
    q = np.asarray(q, np.float32)[perm]
    pn64 = (positions.astype(np.float64) ** 2).sum(1)
    pn = pn64.astype(np.float32)
    pnh, pnl = _split10(pn)
    ph, pl = _split10(positions)
    dmask = 1.0 - np.eye(JCH, dtype=np.float32)

    in_maps = []
    for c in range(NCORES):
        colperm = (np.arange(N) + c * JCH) % N
        lhs = np.zeros((13, N), np.float32)
        lhs[0:3] = -2.0 * ph[colperm].T
        lhs[3:6] = -2.0 * ph[colperm].T
        lhs[6:9] = -2.0 * pl[colperm].T
        lhs[9] = pnh[colperm]
        lhs[10] = pnl[colperm]
        lhs[11] = 1.0
        lhs[12] = 1.0

        # rhs: this core's 8 interleaved row blocks + 4 ghost blocks
        gblocks = [8 * r + c for r in range(8)] + [8 * r + c for r in range(4)]
        ridx = np.concatenate(
            [np.arange(g * JCH, (g + 1) * JCH) for g in gblocks]
        )
        rhs = np.zeros((13, 1536), np.float32)
        rhs[0:3] = ph[ridx].T
        rhs[3:6] = pl[ridx].T
        rhs[6:9] = ph[ridx].T
        rhs[9] = 1.0
        rhs[10] = 1.0
        rhs[11] = pnh[ridx]
        rhs[12] = pnl[ridx]

        qT = np.zeros((JCH, NB * 32), np.float32)
        for p in range(64):
            s, _ = WINDOWS[p]
            atoms = colperm[p * JCH:(p + 1) * JCH]
            qT[:, p * 32 + 4 * s: p * 32 + 4 * s + 4] = q[atoms]

        in_maps.append({"lhs": lhs, "rhs": rhs, "qT": qT, "dmask": dmask})
    return in_maps, positions, q


def _reduce(results, q):
    q64 = np.asarray(q, np.float64)
    pot = 0.0
    for c in range(NCORES):
        F = results[c]["f_out"].astype(np.float64)  # [32, 640]
        Fa = F[:, :512].reshape(8, 4, 512)
        Fe = F[:, 512:640].reshape(8, 4, 128)
        Fc = np.zeros((4, NI), np.float64)
        for s in range(8):
            idx = (np.arange(512) + s * JCH) % NI
            np.add.at(Fc.T, idx, Fa[s].T)
        for s in range(4):
            idx = np.arange(128) + s * JCH + 512
            Fc[:, idx] += Fe[s]
        il = np.arange(NI)
        atoms = (8 * (il // JCH) + c) * JCH + (il % JCH)
        pot += float((q64[atoms].T * Fc).sum())
    pot = pot / TWOPI
    pot += float((q64 ** 2).sum()) / (TWOPI ** 1.5)
    return np.array([pot], dtype=np.float32)


def _run(positions, q, trace=False):
    from concourse.bass_utils import run_bass_kernel_spmd

    perm, meta = _sort_and_flags(np.asarray(positions))
    key = ("nc", meta)
    if key not in _cache:
        _cache[key] = _build(meta)
    nc = _cache[key]
    _cache["nc"] = nc  # for the timing harness
    in_maps, positions, q = _host_inputs(positions, q, perm)
    last_exc = None
    for _attempt in range(3):
        try:
            res = run_bass_kernel_spmd(
                nc, in_maps, core_ids=list(range(NCORES)), trace=trace
            )
            return _reduce(res.results, q), res
        except Exception as exc:  # transient NRT_EXEC_UNIT flakes recover on retry
            last_exc = exc
    raise last_exc


def kernel(positions, q):
    out, _ = _run(positions, q, trace=False)
    return out


# revision 14
# speedup vs baseline: 1.0099x; 1.0099x over previous
"""Ewald realspace potential on 8 Trainium2 NeuronCores — symmetric version.

pot = sum_ij erf(|r_ij|/sqrt(2))/(|r_ij|+1e-6) * (q_i . q_j) / (4*pi)
      + sum(q^2) / (2*pi)^1.5

The pairwise kernel is symmetric, so each unordered 128x128 block pair is
computed exactly once — half the engine work of the row-tiled baseline.

Partitioning (SPMD-uniform, balanced):
  - Atoms are ordered by reverse Cuthill-McKee on the near-pair graph
    (r < CUT), so near pairs live in a narrow diagonal band of the block
    matrix (bandwidth ~4 blocks of 128).
  - 64 row blocks of 128; core c owns the 8 blocks g with g % 8 == c
    (interleaved). Block pair (a, b) with d = (b - a) mod 64 is computed
    by the core owning a iff d in {1..31}, d == 0 (diag), or d == 32 and
    (a div 8) < 4.  Each core gets exactly 260 block pairs.
  - Per core: 64 column positions p (lhs = all 8192 atoms rolled by
    c*128; position p holds global block (c+p) % 64).  Position p needs a
    CONTIGUOUS window of 4 local row blocks (5 for p in {32,40,48,56}),
    identical across cores.  The kernel computes d2[j=128, i=window] via
    an augmented f32r matmul (Dekker hi/lo split, 13 K-rows — K is free),
    then u = rsqrt(d2+1e-6) on ACT (bf16 out), and for the few near
    sub-blocks (window tails, thanks to RCM banding) w = d2*u, e =
    erf(w/sqrt2), kern = e*u.  Far sub-blocks use kern = u exactly
    (erf saturates to 1.0f beyond r~4.3; classification cut 3.0 is safe
    at rel-err ~4e-5).  All staged values bf16 (~4e-4 pot error, budget
    is 2e-2).
  - Reduce: F[32, 512] PSUM accumulates ALL positions via q stationaries
    [128, 32] zero-padded per window group s (8 groups x 4 channels), so
    a single accumulation region at partition 0 suffices.  Window-ext
    columns (5-block positions) go to F_ext[32, 128].  Diagonal blocks
    are kern-scaled by 0.5 (and diag elements masked to 0) so the host
    can uniformly double: pot = sum q_i.F_i / (2*pi) + self.
  - ACT table discipline: one rsqrt phase then one erf phase (2 table
    loads).  Near-tail w values are staged into contiguous bf16 arenas so
    the erf phase is a handful of wide instructions.
"""

import numpy as np

N = 8192
C = 4
NCORES = 8
JCH = 128                 # atoms per block (partition dim)
NB = 64                   # global 128-blocks
NLB = 8                   # local row blocks per core
NI = 1024                 # rows per core
CUT = 3.0                 # near-pair cut for sort + erf classification
RSQRT_BIAS = 1e-6
SQRT1_2 = float(1.0 / np.sqrt(2.0))
TWOPI = 2.0 * np.pi
ARENA_MAX = 8192          # max erf-arena columns per table-phase batch
NECHUNK = 4               # erf instructions per batch (pipelining)

_cache = {}


def _window_table():
    """Static per-position window: (s, w). Window rows are local blocks
    (s+k) % 8 for k in 0..w-1; the diagonal block, when present
    (p % 8 == 0), is always the LAST window block."""
    wins = []
    for p in range(64):
        rows = [
            r for r in range(8)
            if ((p - 8 * r) % 64) <= 31 or (((p - 8 * r) % 64) == 32 and r < 4)
        ]
        w = len(rows)
        rset = set(rows)
        s = next(
            cand for cand in range(8)
            if all(((cand + k) % 8) in rset for k in range(w))
        )
        if p % 8 == 0:
            assert (s + w - 1) % 8 == (p // 8) % 8
        wins.append((s, w))
    return wins


WINDOWS = _window_table()


def _split10(x):
    """Split f32 array into hi (10-bit mantissa, exact under f32r) + lo."""
    x = np.ascontiguousarray(x, dtype=np.float32)
    b = x.view(np.int32) & np.int32(~0x3FFF)
    hi = b.view(np.float32)
    return hi, (x - hi).astype(np.float32)


def _near_pairs(p64):
    """All index pairs (ii, jj), ii<jj, with |p_i - p_j| < CUT."""
    pn = (p64 ** 2).sum(1)
    out_i, out_j = [], []
    for a0 in range(0, N, 1024):
        d2 = pn[a0:a0 + 1024, None] + pn[None, :] - 2.0 * (p64[a0:a0 + 1024] @ p64.T)
        ii, jj = np.nonzero(d2 < CUT * CUT)
        ii = ii + a0
        keep = ii < jj
        out_i.append(ii[keep])
        out_j.append(jj[keep])
    return np.concatenate(out_i), np.concatenate(out_j)


def _rcm_order(p64):
    """Reverse Cuthill-McKee ordering of the near-pair graph (bandwidth
    minimization -> near block pairs concentrate at small block-index
    distance).  scipy if available, else a deterministic numpy BFS RCM."""
    ii, jj = _near_pairs(p64)
    try:
        import scipy.sparse as sp
        from scipy.sparse.csgraph import reverse_cuthill_mckee

        g = sp.csr_matrix(
            (np.ones(len(ii), np.int8), (ii, jj)), shape=(N, N)
        )
        g = g + g.T
        return np.asarray(reverse_cuthill_mckee(g, symmetric_mode=True), np.int64)
    except Exception:
        pass
    # numpy RCM fallback
    order = np.argsort(np.concatenate([ii, jj]), kind="stable")
    src = np.concatenate([ii, jj])[order]
    dst = np.concatenate([jj, ii])[order]
    deg = np.bincount(src, minlength=N)
    starts = np.zeros(N + 1, np.int64)
    np.cumsum(deg, out=starts[1:])
    visited = np.zeros(N, bool)
    out = []
    remaining = set(range(N))
    while remaining:
        root = min(remaining, key=lambda v: (deg[v], v))
        visited[root] = True
        remaining.discard(root)
        queue = [root]
        out.append(root)
        qi = 0
        while qi < len(queue):
            v = queue[qi]
            qi += 1
            nbrs = dst[starts[v]:starts[v + 1]]
            nbrs = [u for u in nbrs.tolist() if not visited[u]]
            nbrs.sort(key=lambda u: (deg[u], u))
            for u in nbrs:
                if not visited[u]:
                    visited[u] = True
                    remaining.discard(u)
                    queue.append(u)
                    out.append(u)
    return np.asarray(out[::-1], np.int64)


def _sort_and_flags(positions):
    """RCM atom order + per-position erf tail start (block index, -1 if the
    position needs no erf at all).  A window sub-block (p, k) is near iff
    ANY core's corresponding global block pair has a pair under CUT (the
    SPMD program is shared, so flags are the union over cores)."""
    p64 = np.asarray(positions, np.float64)
    perm = _rcm_order(p64)
    ps = p64[perm]
    pn = (ps ** 2).sum(1)
    B = np.zeros((NB, NB), dtype=bool)
    for a0 in range(0, N, 1024):
        d2 = pn[a0:a0 + 1024, None] + pn[None, :] - 2.0 * (ps[a0:a0 + 1024] @ ps.T)
        nb = (d2 < CUT * CUT).reshape(8, JCH, NB, JCH).any(axis=(1, 3))
        B[a0 // JCH: a0 // JCH + 8] |= nb
    B |= B.T
    fl_k0 = []
    for p in range(64):
        s, w = WINDOWS[p]
        ks = [
            k for k in range(w)
            if any(B[8 * ((s + k) % 8) + c, (c + p) % 64] for c in range(NCORES))
        ]
        fl_k0.append(min(ks) if ks else -1)
    # diagonal positions must always take the erf path (self-block pairs
    # are near by construction; guard against numeric edge cases)
    for p in range(0, 64, 8):
        s, w = WINDOWS[p]
        if fl_k0[p] < 0:
            fl_k0[p] = w - 1
    return perm, tuple(fl_k0)


def _schedule(meta):
    """Emission schedule: rsqrt pair items + erf chunk assignment.

    Returns (batches, n_ext_total). Each batch:
      items: list of position tuples (1 or 2 positions, same d2 tile)
      echunks: list of lists of flagged positions (one erf inst each)
    """
    ext = [p for p in range(64) if WINDOWS[p][1] == 5]
    reg = [p for p in range(64) if WINDOWS[p][1] == 4]
    regf = [p for p in reg if meta[p] >= 0]
    regu = [p for p in reg if meta[p] < 0]

    def fl_len(p):
        s, w = WINDOWS[p]
        return (w - meta[p]) * JCH if meta[p] >= 0 else 0

    # alternate flagged/unflagged positions, then group into triples so one
    # rsqrt instruction covers 3 positions (d2 tile [128, 1536] = 3 PSUM
    # banks); the 4 extended (640-wide) positions ride as solo items
    inter = []
    fi, ui = 0, 0
    for k in range(len(reg)):
        if (k % 2 == 0 and ui < len(regu)) or fi >= len(regf):
            inter.append(regu[ui]); ui += 1
        else:
            inter.append(regf[fi]); fi += 1
    items = [tuple(inter[i:i + 3]) for i in range(0, len(inter), 3)]
    sx = [(p,) for p in ext]
    step = max(1, len(items) // (len(sx) + 1))
    for i, it in enumerate(sx):
        items.insert(min(len(items), (i + 1) * step + i), it)

    # batches bounded by arena size
    batches = []
    cur, cur_arena = [], 0
    for it in items:
        alen = sum(fl_len(p) for p in it)
        if cur and cur_arena + alen > ARENA_MAX:
            batches.append(cur)
            cur, cur_arena = [], 0
        cur.append(it)
        cur_arena += alen
    if cur:
        batches.append(cur)

    out = []
    for bitems in batches:
        flagged = [p for it in bitems for p in it if meta[p] >= 0]
        total = sum(fl_len(p) for p in flagged)
        nch = min(NECHUNK, max(1, len(flagged)))
        target = max(1, (total + nch - 1) // nch)
        echunks, cur, acc = [], [], 0
        for p in flagged:
            cur.append(p)
            acc += fl_len(p)
            if acc >= target and len(echunks) < nch - 1:
                echunks.append(cur)
                cur, acc = [], 0
        if cur:
            echunks.append(cur)
        out.append((bitems, echunks))
    return out, len(ext)


def _build(meta):
    """meta: tuple of 64 ints — per-position erf tail start block (-1 = no
    erf; kern = rsqrt everywhere in that window)."""
    import concourse.bass as bass
    import concourse.mybir as mybir
    import concourse.tile as tile

    AF = mybir.ActivationFunctionType
    dt = mybir.dt
    ALU = mybir.AluOpType
    nc = bass.Bass(trn_type="TRN2")

    lhs = nc.dram_tensor("lhs", [13, N], dt.float32r, kind="ExternalInput")
    rhs = nc.dram_tensor("rhs", [13, 1536], dt.float32r, kind="ExternalInput")
    qT = nc.dram_tensor("qT", [JCH, NB * 32], dt.float32, kind="ExternalInput")
    dmask = nc.dram_tensor("dmask", [JCH, JCH], dt.float32, kind="ExternalInput")
    f_out = nc.dram_tensor("f_out", [32, 640], dt.float32, kind="ExternalOutput")

    def raw_act(out, in_, func, bias=0.0, scale=1.0):
        return nc.scalar.add_instruction(
            mybir.InstActivation(
                name=nc.get_next_instruction_name(),
                ins=[
                    nc.scalar.lower_ap(in_),
                    mybir.ImmediateValue(dtype=dt.float32, value=bias),
                    mybir.ImmediateValue(dtype=dt.float32, value=scale),
                    mybir.ImmediateValue(dtype=dt.float32, value=0.0),
                ],
                outs=[nc.scalar.lower_ap(out)],
                func=func,
            )
        )

    batches, n_ext_total = _schedule(meta)

    with tile.TileContext(nc) as tc:
        with (
            tc.tile_pool(name="const", bufs=1) as cpool,
            tc.tile_pool(name="u", bufs=1) as upool,
            tc.tile_pool(name="wk", bufs=1) as wpool,
            tc.tile_pool(name="d2", bufs=2, space="PSUM") as d2pool,
            tc.tile_pool(name="facc", bufs=1, space="PSUM") as fpool,
        ):
            lhs_t = cpool.tile([13, N], dt.float32r, tag="lhs")
            rhs_t = cpool.tile([13, 1536], dt.float32r, tag="rhs")
            qf_t = cpool.tile([JCH, NB * 32], dt.float32, tag="qT")
            qb_t = cpool.tile([JCH, NB * 32], dt.bfloat16, tag="qTb")
            m_t = cpool.tile([JCH, JCH], dt.float32, tag="dmask")
            # spread the big lhs load over the 3 DMA-capable engine queues
            # (SP, ACT, Pool), ordered so early positions' data lands first
            nc.scalar.dma_start(lhs_t[:, 0:1024], lhs[:, 0:1024])
            nc.sync.dma_start(rhs_t[:], rhs[:])
            nc.sync.dma_start(lhs_t[:, 1024:3072], lhs[:, 1024:3072])
            nc.gpsimd.dma_start(m_t[:], dmask[:])
            nc.gpsimd.dma_start(qf_t[:], qT[:])
            nc.gpsimd.dma_start(lhs_t[:, 3072:5120], lhs[:, 3072:5120])
            nc.gpsimd.dma_start(lhs_t[:, 5120:8192], lhs[:, 5120:8192])
            nc.vector.tensor_copy(qb_t[:], qf_t[:])

            f_all = fpool.tile([32, 640], dt.float32, tag="fa")

            n_main = [0]
            n_ext = [0]

            def reduce_pos(p, u_ap, W):
                nc.tensor.matmul(
                    f_all[:, 0:512],
                    qb_t[:, p * 32:(p + 1) * 32],
                    u_ap[:, 0:512],
                    start=(n_main[0] == 0),
                    stop=(n_main[0] == 63),
                )
                n_main[0] += 1
                if W > 512:
                    nc.tensor.matmul(
                        f_all[:, 512:512 + (W - 512)],
                        qb_t[:, p * 32:(p + 1) * 32],
                        u_ap[:, 512:W],
                        start=(n_ext[0] == 0),
                        stop=(n_ext[0] == n_ext_total - 1),
                    )
                    n_ext[0] += 1

            prev_last_erf = None
            uidx = [0]
            for bitems, echunks in batches:
                # ---- phase A: d2 matmuls + rsqrt (+ stage w for erf tails,
                # reduce erf-free positions) ----
                pos_u = {}       # p -> (u_ap slice, W)
                warena = {}      # p -> (w_tile, e_tile, offset)
                last_rsqrt = None
                pending_red = []  # delayed phase-A reduces: [(item_idx, p)]
                # pre-alloc per-chunk w/e arenas
                chunk_tiles = []
                for ci, ch in enumerate(echunks):
                    clen = sum(
                        (WINDOWS[p][1] - meta[p]) * JCH for p in ch
                    )
                    w_t = wpool.tile([JCH, clen], dt.bfloat16,
                                     tag=f"w{uidx[0]}_{ci}")
                    e_t = wpool.tile([JCH, clen], dt.bfloat16,
                                     tag=f"e{uidx[0]}_{ci}")
                    off = 0
                    for p in ch:
                        warena[p] = (w_t, e_t, off)
                        off += (WINDOWS[p][1] - meta[p]) * JCH
                    chunk_tiles.append((w_t, e_t))

                for it_i, it in enumerate(bitems):
                    # flush reduces whose u has been ready for >= 2 items, so
                    # they never head-of-line-block the next d2 matmuls in
                    # the PE queue
                    while pending_red and pending_red[0][0] <= it_i - 2:
                        _, rp = pending_red.pop(0)
                        reduce_pos(rp, *pos_u[rp])
                    Ws = [WINDOWS[p][1] * JCH for p in it]
                    tot = sum(Ws)
                    d2 = d2pool.tile([JCH, 1536], dt.float32, tag="d2")
                    off = 0
                    for p, W in zip(it, Ws):
                        s = WINDOWS[p][0]
                        for h0 in range(0, W, 512):
                            h1 = min(h0 + 512, W)
                            nc.tensor.matmul(
                                d2[:, off + h0:off + h1],
                                lhs_t[:, p * JCH:(p + 1) * JCH],
                                rhs_t[:, s * JCH + h0:s * JCH + h1],
                                start=True,
                                stop=True,
                            )
                        if p % 8 == 0:
                            # GpSimd cannot access PSUM, so this sits on DVE
                            sl = slice(off + W - JCH, off + W)
                            nc.vector.tensor_mul(d2[:, sl], d2[:, sl], m_t[:])
                        off += W
                    u_t = upool.tile([JCH, tot], dt.bfloat16,
                                     tag=f"u{uidx[0]}")
                    uidx[0] += 1
                    ri = raw_act(u_t[:], d2[:, 0:tot], AF.Rsqrt,
                                 bias=RSQRT_BIAS)
                    if prev_last_erf is not None:
                        tile.add_dep_helper(
                            ri.ins, prev_last_erf.ins, sync=False,
                            reason="ACT table phase ordering",
                        )
                        prev_last_erf = None
                    last_rsqrt = ri
                    off = 0
                    for p, W in zip(it, Ws):
                        u_ap = u_t[:, off:off + W]
                        pos_u[p] = (u_ap, W)
                        if meta[p] >= 0:
                            w_t, e_t, aoff = warena[p]
                            f0 = meta[p] * JCH
                            nc.vector.tensor_mul(
                                w_t[:, aoff:aoff + W - f0],
                                d2[:, off + f0:off + W],
                                u_ap[:, f0:W],
                            )
                        else:
                            pending_red.append((it_i, p))
                        off += W
                for _, rp in pending_red:
                    reduce_pos(rp, *pos_u[rp])

                # ---- phase B: erf + kern muls + remaining reduces ----
                nmul = [0]
                for ci, ch in enumerate(echunks):
                    w_t, e_t = chunk_tiles[ci]
                    ei = raw_act(e_t[:], w_t[:], AF.Erf, scale=SQRT1_2)
                    tile.add_dep_helper(
                        ei.ins, last_rsqrt.ins, sync=False,
                        reason="ACT table phase ordering (erf after rsqrt)",
                    )
                    prev_last_erf = ei
                    for p in ch:
                        u_ap, W = pos_u[p]
                        _, _, aoff = warena[p]
                        f0 = meta[p] * JCH
                        fl = W - f0
                        if p % 8 == 0:
                            if fl > JCH:
                                eng = nc.gpsimd if nmul[0] % 3 == 2 else nc.vector
                                nmul[0] += 1
                                eng.tensor_mul(
                                    u_ap[:, f0:W - JCH],
                                    e_t[:, aoff:aoff + fl - JCH],
                                    u_ap[:, f0:W - JCH],
                                )
                            # diagonal block: kern *= 0.5 so the host can
                            # uniformly double off-diagonal coverage
                            nc.vector.scalar_tensor_tensor(
                                u_ap[:, W - JCH:W],
                                e_t[:, aoff + fl - JCH:aoff + fl],
                                0.5,
                                u_ap[:, W - JCH:W],
                                ALU.mult,
                                ALU.mult,
                            )
                        else:
                            eng = nc.gpsimd if nmul[0] % 3 == 2 else nc.vector
                            nmul[0] += 1
                            eng.tensor_mul(
                                u_ap[:, f0:W],
                                e_t[:, aoff:aoff + fl],
                                u_ap[:, f0:W],
                            )
                        reduce_pos(p, u_ap, W)

            f_sb = cpool.tile([32, 640], dt.float32, tag="fsb")
            nc.vector.tensor_copy(f_sb[:], f_all[:])
            nc.sync.dma_start(f_out[:], f_sb[:])

    _split_excess_waits(nc)
    return nc


def _split_excess_waits(nc, limit=1):
    """This walrus build accepts at most one sync wait per instruction;
    split extras onto preceding single-wait NOPs on the same engine."""
    import concourse.mybir as mybir

    for f in nc.m.functions:
        for bb in f.blocks:
            new_insts = []
            for inst in bb.instructions:
                si = getattr(inst, "sync_info", None)
                if si is not None and si.on_wait and len(si.on_wait) > limit:
                    waits = list(si.on_wait)
                    extra, keep = waits[:-limit], waits[-limit:]
                    for k, w in enumerate(extra):
                        nop = mybir.InstNoOp(
                            name=f"{inst.name}-ws{k}",
                            ins=[],
                            outs=[],
                            engine=inst.engine,
                            sync_info=mybir.SyncInfo(on_wait=[w], on_update=[]),
                        )
                        nc.register_instruction(nop, overwrite=True)
                        new_insts.append(nop)
                    inst.sync_info = mybir.SyncInfo(
                        on_wait=keep, on_update=list(si.on_update)
                    )
                new_insts.append(inst)
            bb.instructions[:] = new_insts


def _host_inputs(positions, q, perm):
    """Per-core input dicts for the symmetric layout."""
    positions = np.asarray(positions, np.float32)[perm]
    q = np.asarray(q, np.float32)[perm]
    pn64 = (positions.astype(np.float64) ** 2).sum(1)
    pn = pn64.astype(np.float32)
    pnh, pnl = _split10(pn)
    ph, pl = _split10(positions)
    dmask = 1.0 - np.eye(JCH, dtype=np.float32)

    in_maps = []
    for c in range(NCORES):
        colperm = (np.arange(N) + c * JCH) % N
        lhs = np.zeros((13, N), np.float32)
        lhs[0:3] = -2.0 * ph[colperm].T
        lhs[3:6] = -2.0 * ph[colperm].T
        lhs[6:9] = -2.0 * pl[colperm].T
        lhs[9] = pnh[colperm]
        lhs[10] = pnl[colperm]
        lhs[11] = 1.0
        lhs[12] = 1.0

        # rhs: this core's 8 interleaved row blocks + 4 ghost blocks
        gblocks = [8 * r + c for r in range(8)] + [8 * r + c for r in range(4)]
        ridx = np.concatenate(
            [np.arange(g * JCH, (g + 1) * JCH) for g in gblocks]
        )
        rhs = np.zeros((13, 1536), np.float32)
        rhs[0:3] = ph[ridx].T
        rhs[3:6] = pl[ridx].T
        rhs[6:9] = ph[ridx].T
        rhs[9] = 1.0
        rhs[10] = 1.0
        rhs[11] = pnh[ridx]
        rhs[12] = pnl[ridx]

        qT = np.zeros((JCH, NB * 32), np.float32)
        for p in range(64):
            s, _ = WINDOWS[p]
            atoms = colperm[p * JCH:(p + 1) * JCH]
            qT[:, p * 32 + 4 * s: p * 32 + 4 * s + 4] = q[atoms]

        in_maps.append({"lhs": lhs, "rhs": rhs, "qT": qT, "dmask": dmask})
    return in_maps, positions, q


def _reduce(results, q):
    q64 = np.asarray(q, np.float64)
    pot = 0.0
    for c in range(NCORES):
        F = results[c]["f_out"].astype(np.float64)  # [32, 640]
        Fa = F[:, :512].reshape(8, 4, 512)
        Fe = F[:, 512:640].reshape(8, 4, 128)
        Fc = np.zeros((4, NI), np.float64)
        for s in range(8):
            idx = (np.arange(512) + s * JCH) % NI
            np.add.at(Fc.T, idx, Fa[s].T)
        for s in range(4):
            idx = np.arange(128) + s * JCH + 512
            Fc[:, idx] += Fe[s]
        il = np.arange(NI)
        atoms = (8 * (il // JCH) + c) * JCH + (il % JCH)
        pot += float((q64[atoms].T * Fc).sum())
    pot = pot / TWOPI
    pot += float((q64 ** 2).sum()) / (TWOPI ** 1.5)
    return np.array([pot], dtype=np.float32)


def _run(positions, q, trace=False):
    from concourse.bass_utils import run_bass_kernel_spmd

    perm, meta = _sort_and_flags(np.asarray(positions))
    key = ("nc", meta)
    if key not in _cache:
        _cache[key] = _build(meta)
    nc = _cache[key]
    _cache["nc"] = nc  # for the timing harness
    in_maps, positions, q = _host_inputs(positions, q, perm)
    last_exc = None
    for _attempt in range(3):
        try:
            res = run_bass_kernel_spmd(
                nc, in_maps, core_ids=list(range(NCORES)), trace=trace
            )
            return _reduce(res.results, q), res
        except Exception as exc:  # transient NRT_EXEC_UNIT flakes recover on retry
            last_exc = exc
    raise last_exc


def kernel(positions, q):
    out, _ = _run(positions, q, trace=False)
    return out


# revision 17
# speedup vs baseline: 1.0757x; 1.0652x over previous
"""Ewald realspace potential on 8 Trainium2 NeuronCores — symmetric version.

pot = sum_ij erf(|r_ij|/sqrt(2))/(|r_ij|+1e-6) * (q_i . q_j) / (4*pi)
      + sum(q^2) / (2*pi)^1.5

The pairwise kernel is symmetric, so each unordered 128x128 block pair is
computed exactly once — half the engine work of the row-tiled baseline.

Partitioning (SPMD-uniform, balanced):
  - Atoms are ordered by reverse Cuthill-McKee on the near-pair graph
    (r < CUT), so near pairs live in a narrow diagonal band of the block
    matrix (bandwidth ~4 blocks of 128).
  - 64 row blocks of 128; core c owns the 8 blocks g with g % 8 == c
    (interleaved). Block pair (a, b) with d = (b - a) mod 64 is computed
    by the core owning a iff d in {1..31}, d == 0 (diag), or d == 32 and
    (a div 8) < 4.  Each core gets exactly 260 block pairs.
  - Per core: 64 column positions p (lhs = all 8192 atoms rolled by
    c*128; position p holds global block (c+p) % 64).  Position p needs a
    CONTIGUOUS window of 4 local row blocks (5 for p in {32,40,48,56}),
    identical across cores.  The kernel computes d2[j=128, i=window] via
    an augmented f32r matmul (Dekker hi/lo split, 13 K-rows — K is free),
    then u = rsqrt(d2+1e-6) on ACT (bf16 out), and for the few near
    sub-blocks (window tails, thanks to RCM banding) w = d2*u, e =
    erf(w/sqrt2), kern = e*u.  Far sub-blocks use kern = u exactly
    (erf saturates to 1.0f beyond r~4.3; classification cut 3.0 is safe
    at rel-err ~4e-5).  All staged values bf16 (~4e-4 pot error, budget
    is 2e-2).
  - Reduce: F[32, 512] PSUM accumulates ALL positions via q stationaries
    [128, 32] zero-padded per window group s (8 groups x 4 channels), so
    a single accumulation region at partition 0 suffices.  Window-ext
    columns (5-block positions) go to F_ext[32, 128].  Diagonal blocks
    are kern-scaled by 0.5 (and diag elements masked to 0) so the host
    can uniformly double: pot = sum q_i.F_i / (2*pi) + self.
  - ACT table discipline: one rsqrt phase then one erf phase (2 table
    loads).  Near-tail w values are staged into contiguous bf16 arenas so
    the erf phase is a handful of wide instructions.
"""

import numpy as np

N = 8192
C = 4
NCORES = 8
JCH = 128                 # atoms per block (partition dim)
NB = 64                   # global 128-blocks
NLB = 8                   # local row blocks per core
NI = 1024                 # rows per core
CUT = 3.0                 # near-pair cut for sort + erf classification
RSQRT_BIAS = 1e-6
SQRT1_2 = float(1.0 / np.sqrt(2.0))
TWOPI = 2.0 * np.pi
ARENA_MAX = 8192          # max erf-arena columns per table-phase batch
NECHUNK = 4               # erf instructions per batch (pipelining)

_cache = {}


def _window_table():
    """Static per-position window: (s, w). Window rows are local blocks
    (s+k) % 8 for k in 0..w-1; the diagonal block, when present
    (p % 8 == 0), is always the LAST window block."""
    wins = []
    for p in range(64):
        rows = [
            r for r in range(8)
            if ((p - 8 * r) % 64) <= 31 or (((p - 8 * r) % 64) == 32 and r < 4)
        ]
        w = len(rows)
        rset = set(rows)
        s = next(
            cand for cand in range(8)
            if all(((cand + k) % 8) in rset for k in range(w))
        )
        if p % 8 == 0:
            assert (s + w - 1) % 8 == (p // 8) % 8
        wins.append((s, w))
    return wins


WINDOWS = _window_table()


def _split10(x):
    """Split f32 array into hi (10-bit mantissa, exact under f32r) + lo."""
    x = np.ascontiguousarray(x, dtype=np.float32)
    b = x.view(np.int32) & np.int32(~0x3FFF)
    hi = b.view(np.float32)
    return hi, (x - hi).astype(np.float32)


def _near_pairs(p64):
    """All index pairs (ii, jj), ii<jj, with |p_i - p_j| < CUT."""
    pn = (p64 ** 2).sum(1)
    out_i, out_j = [], []
    for a0 in range(0, N, 1024):
        d2 = pn[a0:a0 + 1024, None] + pn[None, :] - 2.0 * (p64[a0:a0 + 1024] @ p64.T)
        ii, jj = np.nonzero(d2 < CUT * CUT)
        ii = ii + a0
        keep = ii < jj
        out_i.append(ii[keep])
        out_j.append(jj[keep])
    return np.concatenate(out_i), np.concatenate(out_j)


def _rcm_order(p64):
    """Reverse Cuthill-McKee ordering of the near-pair graph (bandwidth
    minimization -> near block pairs concentrate at small block-index
    distance).  scipy if available, else a deterministic numpy BFS RCM."""
    ii, jj = _near_pairs(p64)
    try:
        import scipy.sparse as sp
        from scipy.sparse.csgraph import reverse_cuthill_mckee

        g = sp.csr_matrix(
            (np.ones(len(ii), np.int8), (ii, jj)), shape=(N, N)
        )
        g = g + g.T
        return np.asarray(reverse_cuthill_mckee(g, symmetric_mode=True), np.int64)
    except Exception:
        pass
    # numpy RCM fallback
    order = np.argsort(np.concatenate([ii, jj]), kind="stable")
    src = np.concatenate([ii, jj])[order]
    dst = np.concatenate([jj, ii])[order]
    deg = np.bincount(src, minlength=N)
    starts = np.zeros(N + 1, np.int64)
    np.cumsum(deg, out=starts[1:])
    visited = np.zeros(N, bool)
    out = []
    remaining = set(range(N))
    while remaining:
        root = min(remaining, key=lambda v: (deg[v], v))
        visited[root] = True
        remaining.discard(root)
        queue = [root]
        out.append(root)
        qi = 0
        while qi < len(queue):
            v = queue[qi]
            qi += 1
            nbrs = dst[starts[v]:starts[v + 1]]
            nbrs = [u for u in nbrs.tolist() if not visited[u]]
            nbrs.sort(key=lambda u: (deg[u], u))
            for u in nbrs:
                if not visited[u]:
                    visited[u] = True
                    remaining.discard(u)
                    queue.append(u)
                    out.append(u)
    return np.asarray(out[::-1], np.int64)


def _sort_and_flags(positions):
    """RCM atom order + per-position erf tail start (block index, -1 if the
    position needs no erf at all).  A window sub-block (p, k) is near iff
    ANY core's corresponding global block pair has a pair under CUT (the
    SPMD program is shared, so flags are the union over cores)."""
    p64 = np.asarray(positions, np.float64)
    perm = _rcm_order(p64)
    ps = p64[perm]
    pn = (ps ** 2).sum(1)
    B = np.zeros((NB, NB), dtype=bool)
    for a0 in range(0, N, 1024):
        d2 = pn[a0:a0 + 1024, None] + pn[None, :] - 2.0 * (ps[a0:a0 + 1024] @ ps.T)
        nb = (d2 < CUT * CUT).reshape(8, JCH, NB, JCH).any(axis=(1, 3))
        B[a0 // JCH: a0 // JCH + 8] |= nb
    B |= B.T
    fl_k0 = []
    for p in range(64):
        s, w = WINDOWS[p]
        ks = [
            k for k in range(w)
            if any(B[8 * ((s + k) % 8) + c, (c + p) % 64] for c in range(NCORES))
        ]
        fl_k0.append(min(ks) if ks else -1)
    # diagonal positions must always take the erf path (self-block pairs
    # are near by construction; guard against numeric edge cases)
    for p in range(0, 64, 8):
        s, w = WINDOWS[p]
        if fl_k0[p] < 0:
            fl_k0[p] = w - 1
    return perm, tuple(fl_k0)


def _schedule(meta):
    """Emission schedule: rsqrt pair items + erf chunk assignment.

    Returns (batches, n_ext_total). Each batch:
      items: list of position tuples (1 or 2 positions, same d2 tile)
      echunks: list of lists of flagged positions (one erf inst each)
    """
    ext = [p for p in range(64) if WINDOWS[p][1] == 5]
    reg = [p for p in range(64) if WINDOWS[p][1] == 4]
    regf = [p for p in reg if meta[p] >= 0]
    regu = [p for p in reg if meta[p] < 0]

    def fl_len(p):
        s, w = WINDOWS[p]
        return (w - meta[p]) * JCH if meta[p] >= 0 else 0

    # alternate flagged/unflagged positions, then group into triples so one
    # rsqrt instruction covers 3 positions (d2 tile [128, 1536] = 3 PSUM
    # banks); the 4 extended (640-wide) positions ride as solo items
    inter = []
    fi, ui = 0, 0
    for k in range(len(reg)):
        if (k % 2 == 0 and ui < len(regu)) or fi >= len(regf):
            inter.append(regu[ui]); ui += 1
        else:
            inter.append(regf[fi]); fi += 1
    items = [tuple(inter[i:i + 3]) for i in range(0, len(inter), 3)]
    sx = [(p,) for p in ext]
    step = max(1, len(items) // (len(sx) + 1))
    for i, it in enumerate(sx):
        items.insert(min(len(items), (i + 1) * step + i), it)

    # batches bounded by arena size
    batches = []
    cur, cur_arena = [], 0
    for it in items:
        alen = sum(fl_len(p) for p in it)
        if cur and cur_arena + alen > ARENA_MAX:
            batches.append(cur)
            cur, cur_arena = [], 0
        cur.append(it)
        cur_arena += alen
    if cur:
        batches.append(cur)

    out = []
    for bitems in batches:
        flagged = [p for it in bitems for p in it if meta[p] >= 0]
        total = sum(fl_len(p) for p in flagged)
        nch = min(NECHUNK, max(1, len(flagged)))
        target = max(1, (total + nch - 1) // nch)
        echunks, cur, acc = [], [], 0
        for p in flagged:
            cur.append(p)
            acc += fl_len(p)
            if acc >= target and len(echunks) < nch - 1:
                echunks.append(cur)
                cur, acc = [], 0
        if cur:
            echunks.append(cur)
        out.append((bitems, echunks))
    return out, len(ext)


def _build(meta):
    """meta: tuple of 64 ints — per-position erf tail start block (-1 = no
    erf; kern = rsqrt everywhere in that window)."""
    import concourse.bass as bass
    import concourse.mybir as mybir
    import concourse.tile as tile

    AF = mybir.ActivationFunctionType
    dt = mybir.dt
    ALU = mybir.AluOpType
    nc = bass.Bass(trn_type="TRN2")

    lhs = nc.dram_tensor("lhs", [13, N], dt.float32r, kind="ExternalInput")
    rhs = nc.dram_tensor("rhs", [13, 1536], dt.float32r, kind="ExternalInput")
    qT = nc.dram_tensor("qT", [JCH, NB * 32], dt.float32, kind="ExternalInput")
    dmask = nc.dram_tensor("dmask", [JCH, JCH], dt.float32, kind="ExternalInput")
    f_out = nc.dram_tensor("f_out", [32, 640], dt.float32, kind="ExternalOutput")

    def raw_act(out, in_, func, bias=0.0, scale=1.0):
        return nc.scalar.add_instruction(
            mybir.InstActivation(
                name=nc.get_next_instruction_name(),
                ins=[
                    nc.scalar.lower_ap(in_),
                    mybir.ImmediateValue(dtype=dt.float32, value=bias),
                    mybir.ImmediateValue(dtype=dt.float32, value=scale),
                    mybir.ImmediateValue(dtype=dt.float32, value=0.0),
                ],
                outs=[nc.scalar.lower_ap(out)],
                func=func,
            )
        )

    batches, n_ext_total = _schedule(meta)

    with tile.TileContext(nc) as tc:
        with (
            tc.tile_pool(name="const", bufs=1) as cpool,
            tc.tile_pool(name="u", bufs=1) as upool,
            tc.tile_pool(name="wk", bufs=1) as wpool,
            tc.tile_pool(name="d2", bufs=2, space="PSUM") as d2pool,
            tc.tile_pool(name="facc", bufs=1, space="PSUM") as fpool,
        ):
            lhs_t = cpool.tile([13, N], dt.float32r, tag="lhs")
            rhs_t = cpool.tile([13, 1536], dt.float32r, tag="rhs")
            qf_t = cpool.tile([JCH, NB * 32], dt.float32, tag="qT")
            qb_t = cpool.tile([JCH, NB * 32], dt.bfloat16, tag="qTb")
            m_t = cpool.tile([JCH, JCH], dt.float32, tag="dmask")
            # spread the big lhs load over the 3 DMA-capable engine queues
            # (SP, ACT, Pool), ordered so early positions' data lands first
            nc.scalar.dma_start(lhs_t[:, 0:1024], lhs[:, 0:1024])
            nc.sync.dma_start(rhs_t[:], rhs[:])
            nc.sync.dma_start(lhs_t[:, 1024:3072], lhs[:, 1024:3072])
            nc.gpsimd.dma_start(m_t[:], dmask[:])
            nc.gpsimd.dma_start(qf_t[:], qT[:])
            nc.gpsimd.dma_start(lhs_t[:, 3072:5120], lhs[:, 3072:5120])
            nc.gpsimd.dma_start(lhs_t[:, 5120:8192], lhs[:, 5120:8192])
            nc.vector.tensor_copy(qb_t[:], qf_t[:])

            f_all = fpool.tile([32, 640], dt.float32, tag="fa")

            n_main = [0]
            n_ext = [0]

            def reduce_pos(p, u_ap, W):
                nc.tensor.matmul(
                    f_all[:, 0:512],
                    qb_t[:, p * 32:(p + 1) * 32],
                    u_ap[:, 0:512],
                    start=(n_main[0] == 0),
                    stop=(n_main[0] == 63),
                )
                n_main[0] += 1
                if W > 512:
                    nc.tensor.matmul(
                        f_all[:, 512:512 + (W - 512)],
                        qb_t[:, p * 32:(p + 1) * 32],
                        u_ap[:, 512:W],
                        start=(n_ext[0] == 0),
                        stop=(n_ext[0] == n_ext_total - 1),
                    )
                    n_ext[0] += 1

            prev_last_erf = None
            uidx = [0]
            for bitems, echunks in batches:
                # ---- phase A: d2 matmuls + rsqrt (+ stage w for erf tails,
                # reduce erf-free positions) ----
                pos_u = {}       # p -> (u_ap slice, W)
                warena = {}      # p -> (w_tile, e_tile, offset)
                last_rsqrt = None
                pending_red = []  # delayed phase-A reduces: [(item_idx, p)]
                # pre-alloc per-chunk w/e arenas
                chunk_tiles = []
                for ci, ch in enumerate(echunks):
                    clen = sum(
                        (WINDOWS[p][1] - meta[p]) * JCH for p in ch
                    )
                    w_t = wpool.tile([JCH, clen], dt.float32,
                                     tag=f"w{uidx[0]}_{ci}")
                    e_t = wpool.tile([JCH, clen], dt.bfloat16,
                                     tag=f"e{uidx[0]}_{ci}")
                    off = 0
                    for p in ch:
                        warena[p] = (w_t, e_t, off)
                        off += (WINDOWS[p][1] - meta[p]) * JCH
                    chunk_tiles.append((w_t, e_t))

                for it_i, it in enumerate(bitems):
                    # flush reduces whose u has been ready for >= 2 items, so
                    # they never head-of-line-block the next d2 matmuls in
                    # the PE queue
                    while pending_red and pending_red[0][0] <= it_i - 2:
                        _, rp = pending_red.pop(0)
                        reduce_pos(rp, *pos_u[rp])
                    Ws = [WINDOWS[p][1] * JCH for p in it]
                    tot = sum(Ws)
                    d2 = d2pool.tile([JCH, 1536], dt.float32, tag="d2")
                    off = 0
                    for p, W in zip(it, Ws):
                        s = WINDOWS[p][0]
                        for h0 in range(0, W, 512):
                            h1 = min(h0 + 512, W)
                            nc.tensor.matmul(
                                d2[:, off + h0:off + h1],
                                lhs_t[:, p * JCH:(p + 1) * JCH],
                                rhs_t[:, s * JCH + h0:s * JCH + h1],
                                start=True,
                                stop=True,
                            )
                        if p % 8 == 0:
                            # GpSimd cannot access PSUM, so this sits on DVE
                            sl = slice(off + W - JCH, off + W)
                            nc.vector.tensor_mul(d2[:, sl], d2[:, sl], m_t[:])
                        off += W
                    u_t = upool.tile([JCH, tot], dt.bfloat16,
                                     tag=f"u{uidx[0]}")
                    uidx[0] += 1
                    ri = raw_act(u_t[:], d2[:, 0:tot], AF.Rsqrt,
                                 bias=RSQRT_BIAS)
                    if prev_last_erf is not None:
                        tile.add_dep_helper(
                            ri.ins, prev_last_erf.ins, sync=False,
                            reason="ACT table phase ordering",
                        )
                        prev_last_erf = None
                    last_rsqrt = ri
                    off = 0
                    for p, W in zip(it, Ws):
                        u_ap = u_t[:, off:off + W]
                        pos_u[p] = (u_ap, W)
                        if meta[p] >= 0:
                            # w = 1/u = sqrt(d2 + 1e-6): reads only u (SBUF),
                            # so d2's PSUM banks free right after the rsqrt
                            # and PE never stalls on the DVE staging chain.
                            # The masked diagonal gives w exactly 1e-3
                            # (u = rsqrt(1e-6), exact in bf16); the erf bias
                            # below cancels it so kern_ii stays 0.
                            w_t, e_t, aoff = warena[p]
                            f0 = meta[p] * JCH
                            nc.vector.reciprocal(
                                w_t[:, aoff:aoff + W - f0],
                                u_ap[:, f0:W],
                            )
                        else:
                            pending_red.append((it_i, p))
                        off += W
                for _, rp in pending_red:
                    reduce_pos(rp, *pos_u[rp])

                # ---- phase B: erf + kern muls + remaining reduces ----
                nmul = [0]
                for ci, ch in enumerate(echunks):
                    w_t, e_t = chunk_tiles[ci]
                    ei = raw_act(e_t[:], w_t[:], AF.Erf, scale=SQRT1_2,
                                 bias=-1e-3 * SQRT1_2)
                    tile.add_dep_helper(
                        ei.ins, last_rsqrt.ins, sync=False,
                        reason="ACT table phase ordering (erf after rsqrt)",
                    )
                    prev_last_erf = ei
                    for p in ch:
                        u_ap, W = pos_u[p]
                        _, _, aoff = warena[p]
                        f0 = meta[p] * JCH
                        fl = W - f0
                        if p % 8 == 0:
                            if fl > JCH:
                                eng = nc.gpsimd if nmul[0] % 3 == 2 else nc.vector
                                nmul[0] += 1
                                eng.tensor_mul(
                                    u_ap[:, f0:W - JCH],
                                    e_t[:, aoff:aoff + fl - JCH],
                                    u_ap[:, f0:W - JCH],
                                )
                            # diagonal block: kern *= 0.5 so the host can
                            # uniformly double off-diagonal coverage
                            nc.vector.scalar_tensor_tensor(
                                u_ap[:, W - JCH:W],
                                e_t[:, aoff + fl - JCH:aoff + fl],
                                0.5,
                                u_ap[:, W - JCH:W],
                                ALU.mult,
                                ALU.mult,
                            )
                        else:
                            eng = nc.gpsimd if nmul[0] % 3 == 2 else nc.vector
                            nmul[0] += 1
                            eng.tensor_mul(
                                u_ap[:, f0:W],
                                e_t[:, aoff:aoff + fl],
                                u_ap[:, f0:W],
                            )
                        reduce_pos(p, u_ap, W)

            f_sb = cpool.tile([32, 640], dt.float32, tag="fsb")
            nc.vector.tensor_copy(f_sb[:], f_all[:])
            nc.sync.dma_start(f_out[:], f_sb[:])

    _split_excess_waits(nc)
    return nc


def _split_excess_waits(nc, limit=1):
    """This walrus build accepts at most one sync wait per instruction;
    split extras onto preceding single-wait NOPs on the same engine."""
    import concourse.mybir as mybir

    for f in nc.m.functions:
        for bb in f.blocks:
            new_insts = []
            for inst in bb.instructions:
                si = getattr(inst, "sync_info", None)
                if si is not None and si.on_wait and len(si.on_wait) > limit:
                    waits = list(si.on_wait)
                    extra, keep = waits[:-limit], waits[-limit:]
                    for k, w in enumerate(extra):
                        nop = mybir.InstNoOp(
                            name=f"{inst.name}-ws{k}",
                            ins=[],
                            outs=[],
                            engine=inst.engine,
                            sync_info=mybir.SyncInfo(on_wait=[w], on_update=[]),
                        )
                        nc.register_instruction(nop, overwrite=True)
                        new_insts.append(nop)
                    inst.sync_info = mybir.SyncInfo(
                        on_wait=keep, on_update=list(si.on_update)
                    )
                new_insts.append(inst)
            bb.instructions[:] = new_insts


def _host_inputs(positions, q, perm):
    """Per-core input dicts for the symmetric layout."""
    positions = np.asarray(positions, np.float32)[perm]
    q = np.asarray(q, np.float32)[perm]
    pn64 = (positions.astype(np.float64) ** 2).sum(1)
    pn = pn64.astype(np.float32)
    pnh, pnl = _split10(pn)
    ph, pl = _split10(positions)
    dmask = 1.0 - np.eye(JCH, dtype=np.float32)

    in_maps = []
    for c in range(NCORES):
        colperm = (np.arange(N) + c * JCH) % N
        lhs = np.zeros((13, N), np.float32)
        lhs[0:3] = -2.0 * ph[colperm].T
        lhs[3:6] = -2.0 * ph[colperm].T
        lhs[6:9] = -2.0 * pl[colperm].T
        lhs[9] = pnh[colperm]
        lhs[10] = pnl[colperm]
        lhs[11] = 1.0
        lhs[12] = 1.0

        # rhs: this core's 8 interleaved row blocks + 4 ghost blocks
        gblocks = [8 * r + c for r in range(8)] + [8 * r + c for r in range(4)]
        ridx = np.concatenate(
            [np.arange(g * JCH, (g + 1) * JCH) for g in gblocks]
        )
        rhs = np.zeros((13, 1536), np.float32)
        rhs[0:3] = ph[ridx].T
        rhs[3:6] = pl[ridx].T
        rhs[6:9] = ph[ridx].T
        rhs[9] = 1.0
        rhs[10] = 1.0
        rhs[11] = pnh[ridx]
        rhs[12] = pnl[ridx]

        qT = np.zeros((JCH, NB * 32), np.float32)
        for p in range(64):
            s, _ = WINDOWS[p]
            atoms = colperm[p * JCH:(p + 1) * JCH]
            qT[:, p * 32 + 4 * s: p * 32 + 4 * s + 4] = q[atoms]

        in_maps.append({"lhs": lhs, "rhs": rhs, "qT": qT, "dmask": dmask})
    return in_maps, positions, q


def _reduce(results, q):
    q64 = np.asarray(q, np.float64)
    pot = 0.0
    for c in range(NCORES):
        F = results[c]["f_out"].astype(np.float64)  # [32, 640]
        Fa = F[:, :512].reshape(8, 4, 512)
        Fe = F[:, 512:640].reshape(8, 4, 128)
        Fc = np.zeros((4, NI), np.float64)
        for s in range(8):
            idx = (np.arange(512) + s * JCH) % NI
            np.add.at(Fc.T, idx, Fa[s].T)
        for s in range(4):
            idx = np.arange(128) + s * JCH + 512
            Fc[:, idx] += Fe[s]
        il = np.arange(NI)
        atoms = (8 * (il // JCH) + c) * JCH + (il % JCH)
        pot += float((q64[atoms].T * Fc).sum())
    pot = pot / TWOPI
    pot += float((q64 ** 2).sum()) / (TWOPI ** 1.5)
    return np.array([pot], dtype=np.float32)


def _run(positions, q, trace=False):
    from concourse.bass_utils import run_bass_kernel_spmd

    perm, meta = _sort_and_flags(np.asarray(positions))
    key = ("nc", meta)
    if key not in _cache:
        _cache[key] = _build(meta)
    nc = _cache[key]
    _cache["nc"] = nc  # for the timing harness
    in_maps, positions, q = _host_inputs(positions, q, perm)
    last_exc = None
    for _attempt in range(3):
        try:
            res = run_bass_kernel_spmd(
                nc, in_maps, core_ids=list(range(NCORES)), trace=trace
            )
            return _reduce(res.results, q), res
        except Exception as exc:  # transient NRT_EXEC_UNIT flakes recover on retry
            last_exc = exc
    raise last_exc


def kernel(positions, q):
    out, _ = _run(positions, q, trace=False)
    return out


# revision 27
# speedup vs baseline: 1.0906x; 1.0139x over previous
"""Ewald realspace potential on 8 Trainium2 NeuronCores — symmetric version.

pot = sum_ij erf(|r_ij|/sqrt(2))/(|r_ij|+1e-6) * (q_i . q_j) / (4*pi)
      + sum(q^2) / (2*pi)^1.5

The pairwise kernel is symmetric, so each unordered 128x128 block pair is
computed exactly once — half the engine work of the row-tiled baseline.

Partitioning (SPMD-uniform, balanced):
  - Atoms are ordered by reverse Cuthill-McKee on the near-pair graph
    (r < CUT), so near pairs live in a narrow diagonal band of the block
    matrix (bandwidth ~4 blocks of 128).
  - 64 row blocks of 128; core c owns the 8 blocks g with g % 8 == c
    (interleaved). Block pair (a, b) with d = (b - a) mod 64 is computed
    by the core owning a iff d in {1..31}, d == 0 (diag), or d == 32 and
    (a div 8) < 4.  Each core gets exactly 260 block pairs.
  - Per core: 64 column positions p (lhs = all 8192 atoms rolled by
    c*128; position p holds global block (c+p) % 64).  Position p needs a
    CONTIGUOUS window of 4 local row blocks (5 for p in {32,40,48,56}),
    identical across cores.  The kernel computes d2[j=128, i=window] via
    an augmented f32r matmul (Dekker hi/lo split, 13 K-rows — K is free),
    then u = rsqrt(d2+1e-6) on ACT (bf16 out), and for the few near
    sub-blocks (window tails, thanks to RCM banding) w = d2*u, e =
    erf(w/sqrt2), kern = e*u.  Far sub-blocks use kern = u exactly
    (erf saturates to 1.0f beyond r~4.3; classification cut 3.0 is safe
    at rel-err ~4e-5).  All staged values bf16 (~4e-4 pot error, budget
    is 2e-2).
  - Reduce: F[32, 512] PSUM accumulates ALL positions via q stationaries
    [128, 32] zero-padded per window group s (8 groups x 4 channels), so
    a single accumulation region at partition 0 suffices.  Window-ext
    columns (5-block positions) go to F_ext[32, 128].  Diagonal blocks
    are kern-scaled by 0.5 (and diag elements masked to 0) so the host
    can uniformly double: pot = sum q_i.F_i / (2*pi) + self.
  - ACT table discipline: one rsqrt phase then one erf phase (2 table
    loads).  Near-tail w values are staged into contiguous bf16 arenas so
    the erf phase is a handful of wide instructions.
"""

import numpy as np

N = 8192
C = 4
NCORES = 8
JCH = 128                 # atoms per block (partition dim)
NB = 64                   # global 128-blocks
NLB = 8                   # local row blocks per core
NI = 1024                 # rows per core
CUT = 3.0                 # near-pair cut for sort + erf classification
RSQRT_BIAS = 1e-6
SQRT1_2 = float(1.0 / np.sqrt(2.0))
TWOPI = 2.0 * np.pi
ARENA_MAX = 8192          # max erf-arena columns per table-phase batch
NECHUNK = 4               # erf instructions per batch (pipelining)

_cache = {}


def _window_table():
    """Static per-position window: (s, w). Window rows are local blocks
    (s+k) % 8 for k in 0..w-1; the diagonal block, when present
    (p % 8 == 0), is always the LAST window block."""
    wins = []
    for p in range(64):
        rows = [
            r for r in range(8)
            if ((p - 8 * r) % 64) <= 31 or (((p - 8 * r) % 64) == 32 and r < 4)
        ]
        w = len(rows)
        rset = set(rows)
        s = next(
            cand for cand in range(8)
            if all(((cand + k) % 8) in rset for k in range(w))
        )
        if p % 8 == 0:
            assert (s + w - 1) % 8 == (p // 8) % 8
        wins.append((s, w))
    return wins


WINDOWS = _window_table()


def _split10(x):
    """Split f32 array into hi (10-bit mantissa, exact under f32r) + lo."""
    x = np.ascontiguousarray(x, dtype=np.float32)
    b = x.view(np.int32) & np.int32(~0x3FFF)
    hi = b.view(np.float32)
    return hi, (x - hi).astype(np.float32)


def _near_pairs(p64):
    """All index pairs (ii, jj), ii<jj, with |p_i - p_j| < CUT."""
    pn = (p64 ** 2).sum(1)
    out_i, out_j = [], []
    for a0 in range(0, N, 1024):
        d2 = pn[a0:a0 + 1024, None] + pn[None, :] - 2.0 * (p64[a0:a0 + 1024] @ p64.T)
        ii, jj = np.nonzero(d2 < CUT * CUT)
        ii = ii + a0
        keep = ii < jj
        out_i.append(ii[keep])
        out_j.append(jj[keep])
    return np.concatenate(out_i), np.concatenate(out_j)


def _rcm_order(p64):
    """Reverse Cuthill-McKee ordering of the near-pair graph (bandwidth
    minimization -> near block pairs concentrate at small block-index
    distance).  scipy if available, else a deterministic numpy BFS RCM."""
    ii, jj = _near_pairs(p64)
    try:
        import scipy.sparse as sp
        from scipy.sparse.csgraph import reverse_cuthill_mckee

        g = sp.csr_matrix(
            (np.ones(len(ii), np.int8), (ii, jj)), shape=(N, N)
        )
        g = g + g.T
        return np.asarray(reverse_cuthill_mckee(g, symmetric_mode=True), np.int64)
    except Exception:
        pass
    # numpy RCM fallback
    order = np.argsort(np.concatenate([ii, jj]), kind="stable")
    src = np.concatenate([ii, jj])[order]
    dst = np.concatenate([jj, ii])[order]
    deg = np.bincount(src, minlength=N)
    starts = np.zeros(N + 1, np.int64)
    np.cumsum(deg, out=starts[1:])
    visited = np.zeros(N, bool)
    out = []
    remaining = set(range(N))
    while remaining:
        root = min(remaining, key=lambda v: (deg[v], v))
        visited[root] = True
        remaining.discard(root)
        queue = [root]
        out.append(root)
        qi = 0
        while qi < len(queue):
            v = queue[qi]
            qi += 1
            nbrs = dst[starts[v]:starts[v + 1]]
            nbrs = [u for u in nbrs.tolist() if not visited[u]]
            nbrs.sort(key=lambda u: (deg[u], u))
            for u in nbrs:
                if not visited[u]:
                    visited[u] = True
                    remaining.discard(u)
                    queue.append(u)
                    out.append(u)
    return np.asarray(out[::-1], np.int64)


def _sort_and_flags(positions):
    """RCM atom order + per-position erf tail start (block index, -1 if the
    position needs no erf at all).  A window sub-block (p, k) is near iff
    ANY core's corresponding global block pair has a pair under CUT (the
    SPMD program is shared, so flags are the union over cores)."""
    p64 = np.asarray(positions, np.float64)
    perm = _rcm_order(p64)
    ps = p64[perm]
    pn = (ps ** 2).sum(1)
    B = np.zeros((NB, NB), dtype=bool)
    for a0 in range(0, N, 1024):
        d2 = pn[a0:a0 + 1024, None] + pn[None, :] - 2.0 * (ps[a0:a0 + 1024] @ ps.T)
        nb = (d2 < CUT * CUT).reshape(8, JCH, NB, JCH).any(axis=(1, 3))
        B[a0 // JCH: a0 // JCH + 8] |= nb
    B |= B.T
    fl_k0 = []
    for p in range(64):
        s, w = WINDOWS[p]
        ks = [
            k for k in range(w)
            if any(B[8 * ((s + k) % 8) + c, (c + p) % 64] for c in range(NCORES))
        ]
        fl_k0.append(min(ks) if ks else -1)
    # diagonal positions must always take the erf path (self-block pairs
    # are near by construction; guard against numeric edge cases)
    for p in range(0, 64, 8):
        s, w = WINDOWS[p]
        if fl_k0[p] < 0:
            fl_k0[p] = w - 1
    return perm, tuple(fl_k0)


def _schedule(meta):
    """Emission schedule: rsqrt pair items + erf chunk assignment.

    Returns (batches, n_ext_total). Each batch:
      items: list of position tuples (1 or 2 positions, same d2 tile)
      echunks: list of lists of flagged positions (one erf inst each)
    """
    ext = [p for p in range(64) if WINDOWS[p][1] == 5]
    reg = [p for p in range(64) if WINDOWS[p][1] == 4]
    regf = [p for p in reg if meta[p] >= 0]
    regu = [p for p in reg if meta[p] < 0]

    def fl_len(p):
        s, w = WINDOWS[p]
        return (w - meta[p]) * JCH if meta[p] >= 0 else 0

    # alternate flagged/unflagged positions, then group into triples so one
    # rsqrt instruction covers 3 positions (d2 tile [128, 1536] = 3 PSUM
    # banks); the 4 extended (640-wide) positions ride as solo items
    inter = []
    fi, ui = 0, 0
    for k in range(len(reg)):
        if (k % 2 == 0 and ui < len(regu)) or fi >= len(regf):
            inter.append(regu[ui]); ui += 1
        else:
            inter.append(regf[fi]); fi += 1
    items = [tuple(inter[i:i + 3]) for i in range(0, len(inter), 3)]
    sx = [(p,) for p in ext]
    step = max(1, len(items) // (len(sx) + 1))
    for i, it in enumerate(sx):
        items.insert(min(len(items), (i + 1) * step + i), it)

    # batches bounded by arena size
    batches = []
    cur, cur_arena = [], 0
    for it in items:
        alen = sum(fl_len(p) for p in it)
        if cur and cur_arena + alen > ARENA_MAX:
            batches.append(cur)
            cur, cur_arena = [], 0
        cur.append(it)
        cur_arena += alen
    if cur:
        batches.append(cur)

    out = []
    for bitems in batches:
        flagged = [p for it in bitems for p in it if meta[p] >= 0]
        total = sum(fl_len(p) for p in flagged)
        nch = min(NECHUNK, max(1, len(flagged)))
        target = max(1, (total + nch - 1) // nch)
        echunks, cur, acc = [], [], 0
        for p in flagged:
            cur.append(p)
            acc += fl_len(p)
            if acc >= target and len(echunks) < nch - 1:
                echunks.append(cur)
                cur, acc = [], 0
        if cur:
            echunks.append(cur)
        out.append((bitems, echunks))
    return out, len(ext)


def _build(meta):
    """meta: tuple of 64 ints — per-position erf tail start block (-1 = no
    erf; kern = rsqrt everywhere in that window)."""
    import concourse.bass as bass
    import concourse.mybir as mybir
    import concourse.tile as tile

    AF = mybir.ActivationFunctionType
    dt = mybir.dt
    ALU = mybir.AluOpType
    nc = bass.Bass(trn_type="TRN2")

    lhs = nc.dram_tensor("lhs", [13, N], dt.float32r, kind="ExternalInput")
    rhs = nc.dram_tensor("rhs", [13, 1536], dt.float32r, kind="ExternalInput")
    qT = nc.dram_tensor("qT", [JCH, NB * 32], dt.bfloat16, kind="ExternalInput")
    dmask = nc.dram_tensor("dmask", [JCH, JCH], dt.float32, kind="ExternalInput")
    f_out = nc.dram_tensor("f_out", [32, 640], dt.float32, kind="ExternalOutput")

    def raw_act(out, in_, func, bias=0.0, scale=1.0):
        return nc.scalar.add_instruction(
            mybir.InstActivation(
                name=nc.get_next_instruction_name(),
                ins=[
                    nc.scalar.lower_ap(in_),
                    mybir.ImmediateValue(dtype=dt.float32, value=bias),
                    mybir.ImmediateValue(dtype=dt.float32, value=scale),
                    mybir.ImmediateValue(dtype=dt.float32, value=0.0),
                ],
                outs=[nc.scalar.lower_ap(out)],
                func=func,
            )
        )

    batches, n_ext_total = _schedule(meta)

    with tile.TileContext(nc) as tc:
        with (
            tc.tile_pool(name="const", bufs=1) as cpool,
            tc.tile_pool(name="u", bufs=1) as upool,
            tc.tile_pool(name="wk", bufs=1) as wpool,
            tc.tile_pool(name="d2", bufs=2, space="PSUM") as d2pool,
            tc.tile_pool(name="facc", bufs=1, space="PSUM") as fpool,
        ):
            lhs_t = cpool.tile([13, N], dt.float32r, tag="lhs")
            rhs_t = cpool.tile([13, 1536], dt.float32r, tag="rhs")
            qb_t = cpool.tile([JCH, NB * 32], dt.bfloat16, tag="qTb")
            m_t = cpool.tile([JCH, JCH], dt.float32, tag="dmask")
            # spread the big lhs load over the 3 DMA-capable engine queues
            # (SP, ACT, Pool), ordered so early positions' data lands first
            nc.scalar.dma_start(lhs_t[:, 0:1024], lhs[:, 0:1024])
            nc.sync.dma_start(rhs_t[:], rhs[:])
            nc.sync.dma_start(qb_t[:], qT[:])
            nc.sync.dma_start(lhs_t[:, 1024:3072], lhs[:, 1024:3072])
            nc.gpsimd.dma_start(m_t[:], dmask[:])
            nc.gpsimd.dma_start(lhs_t[:, 3072:5120], lhs[:, 3072:5120])
            nc.gpsimd.dma_start(lhs_t[:, 5120:8192], lhs[:, 5120:8192])

            f_all = fpool.tile([32, 640], dt.float32, tag="fa")

            # PSUM start bookkeeping per 128-col F block (5 blocks; col 512
            # is also a PSUM bank boundary, so runs never cross it)
            fstarted = [False] * 5

            def emit_reduce(p, u_ap, a, b, is_last=False):
                """Accumulate q_p^T kern[p][:, a:b] into f_all[:, a:b],
                splitting at started/unstarted block boundaries so every
                PSUM column's first write carries start=True."""
                blks = list(range(a // JCH, b // JCH))
                runs = []
                for k in blks:
                    if runs and fstarted[runs[-1][0]] == fstarted[k] and k != 4:
                        runs[-1].append(k)
                    else:
                        runs.append([k])
                for ri, run in enumerate(runs):
                    lo, hi = run[0] * JCH, (run[-1] + 1) * JCH
                    nc.tensor.matmul(
                        f_all[:, lo:hi],
                        qb_t[:, p * 32:(p + 1) * 32],
                        u_ap[:, lo:hi],
                        start=not fstarted[run[0]],
                        stop=is_last and ri == len(runs) - 1,
                        skip_group_check=True,
                    )
                    for k in run:
                        fstarted[k] = True

            prev_last_erf = None
            uidx = [0]
            for bitems, echunks in batches:
                # ---- phase A: d2 matmuls + rsqrt (+ stage w for erf tails,
                # reduce erf-free positions) ----
                pos_u = {}       # p -> (u_ap slice, W)
                warena = {}      # p -> (w_tile, e_tile, offset)
                last_rsqrt = None
                pending_red = []  # delayed phase-A reduces: [(item_idx, p)]
                # pre-alloc per-chunk w/e arenas
                chunk_tiles = []
                for ci, ch in enumerate(echunks):
                    clen = sum(
                        (WINDOWS[p][1] - meta[p]) * JCH for p in ch
                    )
                    w_t = wpool.tile([JCH, clen], dt.float32,
                                     tag=f"w{uidx[0]}_{ci}")
                    e_t = wpool.tile([JCH, clen], dt.bfloat16,
                                     tag=f"e{uidx[0]}_{ci}")
                    off = 0
                    for p in ch:
                        warena[p] = (w_t, e_t, off)
                        off += (WINDOWS[p][1] - meta[p]) * JCH
                    chunk_tiles.append((w_t, e_t))

                def flush_prefix(rp):
                    u_ap, W = pos_u[rp]
                    pre = meta[rp] * JCH if meta[rp] >= 0 else W
                    if pre > 0:
                        emit_reduce(rp, u_ap, 0, pre)

                for it_i, it in enumerate(bitems):
                    # flush prefix reduces whose u has been ready for >= 2
                    # items, so they never head-of-line-block the next d2
                    # matmuls in the PE queue
                    while pending_red and pending_red[0][0] <= it_i - 2:
                        flush_prefix(pending_red.pop(0)[1])
                    Ws = [WINDOWS[p][1] * JCH for p in it]
                    tot = sum(Ws)
                    d2 = d2pool.tile([JCH, 1536], dt.float32, tag="d2")
                    off = 0
                    for p, W in zip(it, Ws):
                        s = WINDOWS[p][0]
                        for h0 in range(0, W, 512):
                            h1 = min(h0 + 512, W)
                            nc.tensor.matmul(
                                d2[:, off + h0:off + h1],
                                lhs_t[:, p * JCH:(p + 1) * JCH],
                                rhs_t[:, s * JCH + h0:s * JCH + h1],
                                start=True,
                                stop=True,
                            )
                        if p % 8 == 0:
                            # GpSimd cannot access PSUM, so this sits on DVE
                            sl = slice(off + W - JCH, off + W)
                            nc.vector.tensor_mul(d2[:, sl], d2[:, sl], m_t[:])
                        off += W
                    u_t = upool.tile([JCH, tot], dt.bfloat16,
                                     tag=f"u{uidx[0]}")
                    uidx[0] += 1
                    ri = raw_act(u_t[:], d2[:, 0:tot], AF.Rsqrt,
                                 bias=RSQRT_BIAS)
                    if prev_last_erf is not None:
                        tile.add_dep_helper(
                            ri.ins, prev_last_erf.ins, sync=False,
                            reason="ACT table phase ordering",
                        )
                        prev_last_erf = None
                    last_rsqrt = ri
                    off = 0
                    for p, W in zip(it, Ws):
                        u_ap = u_t[:, off:off + W]
                        pos_u[p] = (u_ap, W)
                        if meta[p] >= 0:
                            # w = 1/u = sqrt(d2 + 1e-6): reads only u (SBUF),
                            # so d2's PSUM banks free right after the rsqrt
                            # and PE never stalls on the DVE staging chain.
                            # The masked diagonal gives w exactly 1e-3
                            # (u = rsqrt(1e-6), exact in bf16); the erf bias
                            # below cancels it so kern_ii stays 0.
                            w_t, e_t, aoff = warena[p]
                            f0 = meta[p] * JCH
                            nc.vector.reciprocal(
                                w_t[:, aoff:aoff + W - f0],
                                u_ap[:, f0:W],
                            )
                        pending_red.append((it_i, p))
                        off += W
                for _, rp in pending_red:
                    flush_prefix(rp)

                # ---- phase B: erf + kern muls + tail reduces ----
                nmul = [0]
                is_last_batch = (bitems, echunks) is batches[-1]
                for ci, ch in enumerate(echunks):
                    last_chunk = ci == len(echunks) - 1
                    w_t, e_t = chunk_tiles[ci]
                    ei = raw_act(e_t[:], w_t[:], AF.Erf, scale=SQRT1_2,
                                 bias=-1e-3 * SQRT1_2)
                    tile.add_dep_helper(
                        ei.ins, last_rsqrt.ins, sync=False,
                        reason="ACT table phase ordering (erf after rsqrt)",
                    )
                    prev_last_erf = ei
                    for p in ch:
                        u_ap, W = pos_u[p]
                        _, _, aoff = warena[p]
                        f0 = meta[p] * JCH
                        fl = W - f0
                        if p % 8 == 0:
                            if fl > JCH:
                                eng = nc.gpsimd if nmul[0] % 3 == 2 else nc.vector
                                nmul[0] += 1
                                eng.tensor_mul(
                                    u_ap[:, f0:W - JCH],
                                    e_t[:, aoff:aoff + fl - JCH],
                                    u_ap[:, f0:W - JCH],
                                )
                            # diagonal block: kern *= 0.5 so the host can
                            # uniformly double off-diagonal coverage
                            nc.vector.scalar_tensor_tensor(
                                u_ap[:, W - JCH:W],
                                e_t[:, aoff + fl - JCH:aoff + fl],
                                0.5,
                                u_ap[:, W - JCH:W],
                                ALU.mult,
                                ALU.mult,
                            )
                        else:
                            eng = nc.gpsimd if nmul[0] % 3 == 2 else nc.vector
                            nmul[0] += 1
                            eng.tensor_mul(
                                u_ap[:, f0:W],
                                e_t[:, aoff:aoff + fl],
                                u_ap[:, f0:W],
                            )
                        emit_reduce(
                            p, u_ap, f0, W,
                            is_last=(is_last_batch and last_chunk
                                     and p == ch[-1]),
                        )

            f_sb = cpool.tile([32, 640], dt.float32, tag="fsb")
            nc.vector.tensor_copy(f_sb[:, 0:320], f_all[:, 0:320])
            nc.sync.dma_start(f_out[:, 0:320], f_sb[:, 0:320])
            nc.vector.tensor_copy(f_sb[:, 320:640], f_all[:, 320:640])
            nc.scalar.dma_start(f_out[:, 320:640], f_sb[:, 320:640])

    _split_excess_waits(nc)
    return nc


def _split_excess_waits(nc, limit=1):
    """This walrus build accepts at most one sync wait per instruction;
    split extras onto preceding single-wait NOPs on the same engine."""
    import concourse.mybir as mybir

    for f in nc.m.functions:
        for bb in f.blocks:
            new_insts = []
            for inst in bb.instructions:
                si = getattr(inst, "sync_info", None)
                if si is not None and si.on_wait and len(si.on_wait) > limit:
                    waits = list(si.on_wait)
                    extra, keep = waits[:-limit], waits[-limit:]
                    for k, w in enumerate(extra):
                        nop = mybir.InstNoOp(
                            name=f"{inst.name}-ws{k}",
                            ins=[],
                            outs=[],
                            engine=inst.engine,
                            sync_info=mybir.SyncInfo(on_wait=[w], on_update=[]),
                        )
                        nc.register_instruction(nop, overwrite=True)
                        new_insts.append(nop)
                    inst.sync_info = mybir.SyncInfo(
                        on_wait=keep, on_update=list(si.on_update)
                    )
                new_insts.append(inst)
            bb.instructions[:] = new_insts


def _host_inputs(positions, q, perm):
    """Per-core input dicts for the symmetric layout."""
    positions = np.asarray(positions, np.float32)[perm]
    q = np.asarray(q, np.float32)[perm]
    pn64 = (positions.astype(np.float64) ** 2).sum(1)
    pn = pn64.astype(np.float32)
    pnh, pnl = _split10(pn)
    ph, pl = _split10(positions)
    dmask = 1.0 - np.eye(JCH, dtype=np.float32)

    in_maps = []
    for c in range(NCORES):
        colperm = (np.arange(N) + c * JCH) % N
        lhs = np.zeros((13, N), np.float32)
        lhs[0:3] = -2.0 * ph[colperm].T
        lhs[3:6] = -2.0 * ph[colperm].T
        lhs[6:9] = -2.0 * pl[colperm].T
        lhs[9] = pnh[colperm]
        lhs[10] = pnl[colperm]
        lhs[11] = 1.0
        lhs[12] = 1.0

        # rhs: this core's 8 interleaved row blocks + 4 ghost blocks
        gblocks = [8 * r + c for r in range(8)] + [8 * r + c for r in range(4)]
        ridx = np.concatenate(
            [np.arange(g * JCH, (g + 1) * JCH) for g in gblocks]
        )
        rhs = np.zeros((13, 1536), np.float32)
        rhs[0:3] = ph[ridx].T
        rhs[3:6] = pl[ridx].T
        rhs[6:9] = ph[ridx].T
        rhs[9] = 1.0
        rhs[10] = 1.0
        rhs[11] = pnh[ridx]
        rhs[12] = pnl[ridx]

        qT = np.zeros((JCH, NB * 32), np.float32)
        for p in range(64):
            s, _ = WINDOWS[p]
            atoms = colperm[p * JCH:(p + 1) * JCH]
            qT[:, p * 32 + 4 * s: p * 32 + 4 * s + 4] = q[atoms]
        import ml_dtypes
        qT = qT.astype(ml_dtypes.bfloat16)

        in_maps.append({"lhs": lhs, "rhs": rhs, "qT": qT, "dmask": dmask})
    return in_maps, positions, q


def _reduce(results, q):
    q64 = np.asarray(q, np.float64)
    pot = 0.0
    for c in range(NCORES):
        F = results[c]["f_out"].astype(np.float64)  # [32, 640]
        Fa = F[:, :512].reshape(8, 4, 512)
        Fe = F[:, 512:640].reshape(8, 4, 128)
        Fc = np.zeros((4, NI), np.float64)
        for s in range(8):
            idx = (np.arange(512) + s * JCH) % NI
            np.add.at(Fc.T, idx, Fa[s].T)
        for s in range(4):
            idx = np.arange(128) + s * JCH + 512
            Fc[:, idx] += Fe[s]
        il = np.arange(NI)
        atoms = (8 * (il // JCH) + c) * JCH + (il % JCH)
        pot += float((q64[atoms].T * Fc).sum())
    pot = pot / TWOPI
    pot += float((q64 ** 2).sum()) / (TWOPI ** 1.5)
    return np.array([pot], dtype=np.float32)


def _run(positions, q, trace=False):
    from concourse.bass_utils import run_bass_kernel_spmd

    perm, meta = _sort_and_flags(np.asarray(positions))
    key = ("nc", meta)
    if key not in _cache:
        _cache[key] = _build(meta)
    nc = _cache[key]
    _cache["nc"] = nc  # for the timing harness
    in_maps, positions, q = _host_inputs(positions, q, perm)
    last_exc = None
    for _attempt in range(3):
        try:
            res = run_bass_kernel_spmd(
                nc, in_maps, core_ids=list(range(NCORES)), trace=trace
            )
            return _reduce(res.results, q), res
        except Exception as exc:  # transient NRT_EXEC_UNIT flakes recover on retry
            last_exc = exc
    raise last_exc


def kernel(positions, q):
    out, _ = _run(positions, q, trace=False)
    return out
